# revision 1
# baseline (speedup 1.0000x reference)
"""GCN (3-layer GCNConv + mean-pool + MLP) on 8 Trainium2 NeuronCores.

Strategy (graph/data parallel, per sharding hint):
  - Nodes are partitioned by graph id (batch is sorted) into 8 contiguous
    slices; each core owns the edges whose *destination* is local.
  - Transform-first GCN: per layer, table = h @ W is built node-major in
    DRAM (layer 1 replicated from x on every core; layers 2-3 local slice
    + AllGather), then each core aggregates its local edges by dma_gather
    of source rows and one-hot matmul accumulation into PSUM.
  - The "one-hot" rhs matrices are precomputed ON HOST with the full GCN
    edge normalization dis[src]*dis[dst] baked into the nonzero values,
    and streamed from DRAM via HWDGE.  This keeps DVE completely out of
    the aggregation loop (a DVE op holds the SBUF port pair that the Q7
    SWDGE descriptor generator needs, which starves the gathers).
  - Self-loops are explicit edges with weight dis_i^2.  Layer-3 bias is
    folded into the MLP bias (b4' = b4 + b3 @ W4), so aggregation
    evictions are single ACT instructions.
  - Pooling: PE-transpose of h3 blocks + matmul against a host-built
    (1/cnt)-weighted graph-membership matrix, then the 2-layer MLP.

The int16 gather indices use a signed-base trick: two ranges with bases at
rows 32768 and 98304 so idx in [-32768, 32767] covers up to 131072 rows.
Padding slots use idx 0 (a valid row) with zero one-hot weight, so no
trailing-negative fixups are needed.
"""
import os
import sys

sys.path.insert(0, "/opt/trn_rl_repo")

import numpy as np

from concourse import bacc, bass, mybir, tile
from concourse import library_config
from concourse.bass_utils import run_bass_kernel_spmd
from concourse.masks import make_identity

F32 = mybir.dt.float32
BF16 = mybir.dt.bfloat16
I16 = mybir.dt.int16
NP_BF16 = mybir.dt.np(BF16)

N_CORES = 8
HID = 128
N_CLASSES = 3
SLAB = 1024          # nodes per x_fm load slab in transform-1
STORE_J = 4          # 128-row blocks per table store DMA

GROUP = int(os.environ.get("GNN_GROUP", "256"))      # dst nodes per PSUM group
SG_GROUPS = int(os.environ.get("GNN_SG", "2"))       # groups per gather call

_BUILD_CACHE = {}
LAST_EXEC_NS = None
LAST_TRACE_DIR = None


# ----------------------------------------------------------------- host prep

def _prep(x, edge_index, batch):
    N, F = x.shape
    G = int(batch.max()) + 1 if batch.size else 1

    assert G % N_CORES == 0, G
    gpc = G // N_CORES

    batch = np.asarray(batch, dtype=np.int64)
    src = np.asarray(edge_index[0], dtype=np.int64)
    dst = np.asarray(edge_index[1], dtype=np.int64)

    gstart = np.searchsorted(batch, np.arange(G + 1), side="left")
    starts = gstart[0 : G + 1 : gpc]            # [C+1] node boundaries
    M = np.diff(starts)                          # nodes per core
    Mp = int(np.ceil(M.max() / max(GROUP, 128)) * max(GROUP, 128))
    R = N_CORES * Mp                             # table rows
    assert R <= 131072, R
    NB = Mp // 128                               # local 128-node blocks

    # node -> table row
    owner = np.searchsorted(starts, np.arange(N), side="right") - 1
    row_of = owner * Mp + (np.arange(N) - starts[owner])

    # degree (in-degree over real edges) + 1, as in PyG gcn_norm
    deg = np.bincount(dst, minlength=N).astype(np.float32) + 1.0
    dis = 1.0 / np.sqrt(deg)

    # gather index ranges (signed-base int16)
    if R <= 32768:
        bases = [0]
    else:
        bases = [32768, 98304]
    n_ranges = len(bases)

    ngroups = Mp // GROUP
    nsg = (ngroups + SG_GROUPS - 1) // SG_GROUPS
    nkeys = nsg * n_ranges * SG_GROUPS

    # per-core edge arrays: src table row, local dst, edge weight
    e_owner = owner[dst]
    core_rows, core_d, core_w = [], [], []
    for c in range(N_CORES):
        m = e_owner == c
        s_c, e_c = int(starts[c]), int(starts[c + 1])
        esrc, edst = src[m], dst[m]
        selfn = np.arange(s_c, e_c)
        rows = np.concatenate([row_of[esrc], row_of[selfn]])
        d = np.concatenate([edst - s_c, selfn - s_c])
        w = np.concatenate([dis[esrc] * dis[edst], dis[selfn] * dis[selfn]])
        core_rows.append(rows)
        core_d.append(d)
        core_w.append(w.astype(np.float32))

    def edge_keys(rows, d):
        gidx = d // GROUP
        sg = gidx // SG_GROUPS
        g_in = gidx % SG_GROUPS
        if n_ranges == 1:
            r = np.zeros_like(rows)
        else:
            r = (rows >= 65536).astype(np.int64)
        return (sg * n_ranges + r) * SG_GROUPS + g_in

    # uniform chunk counts per key (max over cores), >=1 chunk per group
    kc = np.zeros((N_CORES, nkeys), np.int64)
    core_keys = []
    for c in range(N_CORES):
        key = edge_keys(core_rows[c], core_d[c])
        core_keys.append(key)
        kc[c] = (np.bincount(key, minlength=nkeys) + 127) // 128
    K = kc.max(axis=0)
    Kv = K.reshape(nsg, n_ranges, SG_GROUPS)
    gtot = Kv.sum(axis=1)
    for sg in range(nsg):
        for g in range(SG_GROUPS):
            if gtot[sg, g] == 0:
                Kv[sg, 0, g] = 1
    K = Kv.reshape(-1)
    T = int(K.sum())
    slot_off = np.zeros(nkeys + 1, np.int64)
    np.cumsum(K * 128, out=slot_off[1:])
    S = int(slot_off[-1])
    S16 = S // 16

    IDX = np.zeros((N_CORES, 128, S16), np.int16)
    OH = np.zeros((N_CORES, 128, T, GROUP), NP_BF16)
    for c in range(N_CORES):
        rows, d, w = core_rows[c], core_d[c], core_w[c]
        key = core_keys[c]
        order = np.argsort(key, kind="stable")
        ks = key[order]
        # position within key run
        if len(ks):
            new = np.empty(len(ks), bool)
            new[0] = True
            new[1:] = ks[1:] != ks[:-1]
            run_start = np.nonzero(new)[0][np.cumsum(new) - 1]
            pos = np.arange(len(ks)) - run_start
        else:
            pos = np.zeros(0, np.int64)
        slot = slot_off[ks] + pos
        rows_o = rows[order]
        if n_ranges == 1:
            i16 = rows_o
        else:
            rr = rows_o >= 65536
            i16 = rows_o - np.where(rr, 98304, 32768)
        idx_slots = np.zeros(S, np.int16)
        idx_slots[slot] = i16.astype(np.int16)
        IDX[c] = np.tile(idx_slots.reshape(S16, 16).T, (8, 1))
        # dis-weighted one-hot, [128 part(slot%128), T chunk, GROUP col]
        ohc = np.zeros((128, T, GROUP), np.float32)
        p = slot % 128
        t = slot // 128
        col = d[order] % GROUP
        ohc[p, t, col] = w[order]
        OH[c] = ohc.astype(NP_BF16)

    # x feature-major in table-row order [F, R], bf16
    X_FM = np.zeros((F, R), np.float32)
    X_FM[:, row_of] = np.asarray(x, np.float32).T
    X_FM = X_FM.astype(NP_BF16)

    # pooling matrix: GO[p, b, g] = 1/cnt_g if local node b*128+p in graph g
    GO = np.zeros((N_CORES, 128, NB, gpc), np.float32)
    for c in range(N_CORES):
        s_c, e_c = int(starts[c]), int(starts[c + 1])
        mloc = e_c - s_c
        bl = (batch[s_c:e_c] - c * gpc).astype(np.int64)
        cnt = np.bincount(bl, minlength=gpc).astype(np.float32)
        inv = 1.0 / np.maximum(cnt, 1.0)
        n = np.arange(mloc)
        GO[c, n % 128, n // 128, bl] = inv[bl]

    meta = dict(
        F=F, R=R, Mp=Mp, T=T, S16=S16, nsg=nsg, ngroups=ngroups, NB=NB,
        n_ranges=n_ranges, bases=tuple(bases), gpc=gpc,
        group=GROUP, sg_groups=SG_GROUPS,
        K=tuple(int(v) for v in K),
        slot_off=tuple(int(v) for v in slot_off),
    )
    in_maps = []
    for c in range(N_CORES):
        in_maps.append({
            "x_fm": X_FM,
            "idx": IDX[c],
            "oh": OH[c],
            "go": GO[c],
        })
    return meta, in_maps


def _weight_inputs(inputs):
    """Weight/bias arrays shared by kernel() and test.py."""
    w4 = np.asarray(inputs["W4"], np.float32)
    b3 = np.asarray(inputs["b3"], np.float32)
    b4p = np.asarray(inputs["b4"], np.float32) + b3 @ w4
    return {
        "w1": np.asarray(inputs["W1"], np.float32).astype(NP_BF16),
        "w2": np.asarray(inputs["W2"], np.float32).astype(NP_BF16),
        "w3": np.asarray(inputs["W3"], np.float32).astype(NP_BF16),
        "w4": w4,
        "w5": np.asarray(inputs["W5"], np.float32),
        "b1": np.broadcast_to(np.asarray(inputs["b1"], np.float32)[:, None], (HID, 1)).copy(),
        "b2": np.broadcast_to(np.asarray(inputs["b2"], np.float32)[:, None], (HID, 1)).copy(),
        "b4p": b4p[None, :],
        "b5": np.asarray(inputs["b5"], np.float32)[None, :],
    }


# --------------------------------------------------------------- device build

def _build(meta):
    F = meta["F"]
    R = meta["R"]
    Mp = meta["Mp"]
    T = meta["T"]
    S16 = meta["S16"]
    nsg = meta["nsg"]
    ngroups = meta["ngroups"]
    NB = meta["NB"]
    n_ranges = meta["n_ranges"]
    bases = meta["bases"]
    gpc = meta["gpc"]
    group = meta["group"]
    sg_groups = meta["sg_groups"]
    K = np.array(meta["K"], np.int64).reshape(nsg, n_ranges, sg_groups)
    slot_off = np.array(meta["slot_off"], np.int64)

    nc = bacc.Bacc("TRN2")

    x_fm = nc.declare_dram_parameter("x_fm", [F, R], BF16, isOutput=False)
    idx_p = nc.declare_dram_parameter("idx", [128, S16], I16, isOutput=False)
    oh_p = nc.declare_dram_parameter("oh", [128, T, group], BF16, isOutput=False)
    go_p = nc.declare_dram_parameter("go", [128, NB, gpc], F32, isOutput=False)
    w1 = nc.declare_dram_parameter("w1", [F, HID], BF16, isOutput=False)
    w2 = nc.declare_dram_parameter("w2", [HID, HID], BF16, isOutput=False)
    w3 = nc.declare_dram_parameter("w3", [HID, HID], BF16, isOutput=False)
    w4 = nc.declare_dram_parameter("w4", [HID, HID // 2], F32, isOutput=False)
    w5 = nc.declare_dram_parameter("w5", [HID // 2, N_CLASSES], F32, isOutput=False)
    b1 = nc.declare_dram_parameter("b1", [HID, 1], F32, isOutput=False)
    b2 = nc.declare_dram_parameter("b2", [HID, 1], F32, isOutput=False)
    b4p = nc.declare_dram_parameter("b4p", [1, HID // 2], F32, isOutput=False)
    b5 = nc.declare_dram_parameter("b5", [1, N_CLASSES], F32, isOutput=False)
    out_p = nc.declare_dram_parameter("out", [N_CLASSES, gpc], F32, isOutput=True)

    tbl1 = nc.dram_tensor("tbl1", [R, HID], BF16)

    maxch = 0
    for sg in range(nsg):
        maxch = max(maxch, int(K[sg].sum()))

    with tile.TileContext(nc) as tc:
        nc.gpsimd.load_library(library_config.mlp)
        with (
            tc.tile_pool(name="const", bufs=1) as constp,
            tc.tile_pool(name="hbuf", bufs=1) as hpool,
            tc.tile_pool(name="gbuf", bufs=2) as gpool,
            tc.tile_pool(name="xslab", bufs=2) as xpool,
            tc.tile_pool(name="ohb", bufs=2) as ohpool,
            tc.tile_pool(name="idxb", bufs=2) as idxpool,
            tc.tile_pool(name="ev", bufs=2) as evpool,
            tc.tile_pool(name="stb", bufs=2) as stpool,
            tc.tile_pool(name="tpsum", bufs=4, space="PSUM") as tpsum,
            tc.tile_pool(name="gpsum", bufs=1, space="PSUM") as gpsum,
            tc.tile_pool(name="ppsum", bufs=1, space="PSUM") as ppsum,
            tc.tile_pool(name="dram", bufs=1, space="DRAM") as dramp,
        ):
            # ---- constants in SBUF
            w_t = {}
            for nm, p, shp, dt in (("w1", w1, [F, HID], BF16),
                                   ("w2", w2, [HID, HID], BF16),
                                   ("w3", w3, [HID, HID], BF16),
                                   ("w4", w4, [HID, HID // 2], F32),
                                   ("w5", w5, [HID // 2, N_CLASSES], F32)):
                w_t[nm] = constp.tile(shp, dt, tag=nm, name=nm)
                nc.sync.dma_start(out=w_t[nm][:], in_=p[:])
            b_t = {}
            for nm, p, shp in (("b1", b1, [HID, 1]), ("b2", b2, [HID, 1]),
                               ("b4p", b4p, [1, HID // 2]), ("b5", b5, [1, N_CLASSES])):
                b_t[nm] = constp.tile(shp, F32, tag=nm, name=nm)
                nc.sync.dma_start(out=b_t[nm][:], in_=p[:])
            go_t = constp.tile([128, NB, gpc], F32, tag="go", name="go")
            nc.sync.dma_start(out=go_t[:], in_=go_p[:])
            ones_t = constp.tile([1, max(gpc, 128)], F32)
            nc.vector.memset(ones_t[:], 1.0)
            ident = constp.tile([128, 128], BF16)
            make_identity(nc, ident[:])

            # ---- transform: tbl[row0+j] = h_src(j).T @ W (node-major bf16)
            # One matmul accumulation group per PSUM bank: a start=True
            # into any slice of a bank resets the whole bank.
            def transform(h_src, W, tbl_out, nblk, row0, split_dve):
                for j0 in range(0, nblk, STORE_J):
                    jn = min(STORE_J, nblk - j0)
                    st = stpool.tile([128, STORE_J, HID], BF16, tag="tst")
                    for j in range(j0, j0 + jn):
                        jj = j - j0
                        ps = tpsum.tile([128, HID], F32, tag="tps", name="tps")
                        nc.tensor.matmul(ps[:], lhsT=h_src(j), rhs=W,
                                         start=True, stop=True)
                        if split_dve and jj % 2 == 1:
                            nc.vector.tensor_copy(out=st[:, jj, :], in_=ps[:])
                        else:
                            nc.scalar.activation(
                                out=st[:, jj, :], in_=ps[:],
                                func=mybir.ActivationFunctionType.Copy)
                    nc.sync.dma_start(
                        out=tbl_out[(row0 + j0) * 128 : (row0 + j0 + jn) * 128, :]
                        .rearrange("(j p) f -> p j f", p=128),
                        in_=st[:, :jn, :],
                    )

            # ---- aggregation: h_out = act(sum_e oh[e] * tbl[src_e]) (+bias)
            def aggregate(tbl_src, h_out, bias_ap, relu):
                for sg in range(nsg):
                    ch_r = [int(K[sg, r].sum()) for r in range(n_ranges)]
                    ch_tot = sum(ch_r)
                    t0 = int(slot_off[sg * n_ranges * sg_groups]) // 128
                    oh_t = ohpool.tile([128, maxch, group], BF16, tag="oh")
                    nc.sync.dma_start(out=oh_t[:, :ch_tot, :],
                                      in_=oh_p[:, t0 : t0 + ch_tot, :])
                    idx_t = idxpool.tile([128, maxch * 8], I16, tag="idx")
                    nc.sync.dma_start(out=idx_t[:, : ch_tot * 8],
                                      in_=idx_p[:, t0 * 8 : (t0 + ch_tot) * 8])
                    gb = gpool.tile([128, maxch, HID], BF16, tag="gb")
                    off = 0
                    for r in range(n_ranges):
                        if ch_r[r] == 0:
                            continue
                        nix = ch_r[r] * 128
                        nc.gpsimd.dma_gather(
                            gb[:, off : off + ch_r[r], :],
                            tbl_src[bases[r] :, :],
                            idx_t[:, off * 8 : off * 8 + nix // 16],
                            nix, nix, HID,
                            single_packet=False,
                            queue_num=0,
                        )
                        off += ch_r[r]
                    ngrp = min(sg_groups, ngroups - sg * sg_groups)
                    gps = [gpsum.tile([128, group], F32, tag=f"gp{i}",
                                      name=f"gp{i}") for i in range(ngrp)]
                    started = [False] * ngrp
                    remaining = [int(K[sg, :, g].sum()) for g in range(ngrp)]
                    ch = 0
                    for r in range(n_ranges):
                        for g in range(ngrp):
                            for t in range(int(K[sg, r, g])):
                                remaining[g] -= 1
                                nc.tensor.matmul(
                                    gps[g][:],
                                    lhsT=gb[:, ch, :], rhs=oh_t[:, ch, :],
                                    start=not started[g],
                                    stop=remaining[g] == 0,
                                )
                                started[g] = True
                                ch += 1
                    for g in range(ngrp):
                        n0 = (sg * sg_groups + g) * group
                        if relu:
                            nc.scalar.activation(
                                out=h_out[:, n0 : n0 + group],
                                in_=gps[g][:],
                                func=mybir.ActivationFunctionType.Relu,
                                bias=bias_ap)
                        else:
                            nc.scalar.activation(
                                out=h_out[:, n0 : n0 + group],
                                in_=gps[g][:],
                                func=mybir.ActivationFunctionType.Copy)

            for _rep in range(int(os.environ.get("GNN_REPS", "1"))):
                # transform-1: full table from x (replicated on every core)
                for s in range(R // SLAB):
                    xs = xpool.tile([F, SLAB], BF16, tag="xs")
                    nc.sync.dma_start(out=xs[:], in_=x_fm[:, s * SLAB : (s + 1) * SLAB])
                    transform(
                        h_src=lambda j, xs=xs: xs[:, j * 128 : (j + 1) * 128],
                        W=w_t["w1"][:], tbl_out=tbl1,
                        nblk=SLAB // 128, row0=s * (SLAB // 128),
                        split_dve=True,
                    )

                h_t = hpool.tile([128, Mp], BF16, tag="h")
                aggregate(tbl1, h_t, b_t["b1"][:, :], relu=True)

                # ---- layers 2 and 3: local transform + AllGather + aggregate
                for lay, (Wl, bl, relu) in enumerate(
                    ((w_t["w2"], b_t["b2"], True), (w_t["w3"], None, False))
                ):
                    cin = dramp.tile([Mp, HID], BF16, tag=f"cin{lay}")
                    cout = dramp.tile([R, HID], BF16, tag=f"cout{lay}",
                                      addr_space="Shared")
                    transform(
                        h_src=lambda j, h=h_t: h[:, j * 128 : (j + 1) * 128],
                        W=Wl[:], tbl_out=cin, nblk=NB, row0=0,
                        split_dve=False,
                    )
                    nc.gpsimd.collective_compute(
                        "AllGather", mybir.AluOpType.bypass,
                        replica_groups=[list(range(N_CORES))],
                        ins=[cin[:]], outs=[cout[:]],
                    )
                    h2_t = hpool.tile([128, Mp], BF16, tag="h")
                    aggregate(cout, h2_t,
                              bl[:, :] if bl is not None else None, relu=relu)
                    h_t = h2_t

                # ---- pooling + MLP (all f32, small)
                pool_ps = ppsum.tile([128, gpc], F32, tag="pp", name="pool_ps")
                for b in range(NB):
                    tp = ppsum.tile([128, 128], BF16, tag="tp", name="tp")
                    nc.tensor.transpose(out=tp[:], in_=h_t[:, b * 128 : (b + 1) * 128],
                                        identity=ident[:])
                    hnm = evpool.tile([128, 128], F32, tag="hnm")
                    nc.scalar.activation(out=hnm[:], in_=tp[:],
                                         func=mybir.ActivationFunctionType.Copy)
                    nc.tensor.matmul(pool_ps[:], lhsT=hnm[:], rhs=go_t[:, b, :],
                                     start=(b == 0), stop=(b == NB - 1))
                pooled = evpool.tile([128, gpc], F32, tag="pooled")
                nc.scalar.activation(out=pooled[:], in_=pool_ps[:],
                                     func=mybir.ActivationFunctionType.Copy)
                zps = ppsum.tile([HID // 2, gpc], F32, tag="pp", name="zps")
                nc.tensor.matmul(zps[:], lhsT=w_t["w4"][:], rhs=pooled[:],
                                 start=True, stop=False)
                nc.tensor.matmul(zps[:], lhsT=b_t["b4p"][:], rhs=ones_t[:, :gpc],
                                 start=False, stop=True)
                z_t = evpool.tile([HID // 2, gpc], F32, tag="z")
                nc.scalar.activation(out=z_t[:], in_=zps[:],
                                     func=mybir.ActivationFunctionType.Relu)
                ops = ppsum.tile([N_CLASSES, gpc], F32, tag="pp", name="ops")
                nc.tensor.matmul(ops[:], lhsT=w_t["w5"][:], rhs=z_t[:],
                                 start=True, stop=False)
                nc.tensor.matmul(ops[:], lhsT=b_t["b5"][:], rhs=ones_t[:, :gpc],
                                 start=False, stop=True)
                o_t = evpool.tile([N_CLASSES, gpc], F32, tag="o")
                nc.scalar.activation(out=o_t[:], in_=ops[:],
                                     func=mybir.ActivationFunctionType.Copy)
                nc.sync.dma_start(out=out_p[:], in_=o_t[:])

    nc.compile()
    return nc


# -------------------------------------------------------------------- kernel

def kernel(**inputs):
    x = np.asarray(inputs["x"], np.float32)
    edge_index = np.asarray(inputs["edge_index"])
    batch = np.asarray(inputs["batch"])
    meta, in_maps = _prep(x, edge_index, batch)

    key = repr(sorted(meta.items()))
    if key not in _BUILD_CACHE:
        _BUILD_CACHE[key] = _build(meta)
    nc = _BUILD_CACHE[key]

    wmap = _weight_inputs(inputs)
    for im in in_maps:
        im.update(wmap)

    trace = bool(os.environ.get("GNN_TRACE"))
    import tempfile
    tdir = tempfile.mkdtemp(prefix="gnn_trace_") if trace else None
    res = run_bass_kernel_spmd(nc, in_maps, list(range(N_CORES)),
                               trace=trace, tmpdir=tdir)
    global LAST_EXEC_NS, LAST_TRACE_DIR
    LAST_EXEC_NS = res.exec_time_ns
    LAST_TRACE_DIR = tdir
    gpc = meta["gpc"]
    G = gpc * N_CORES
    out = np.zeros((G, N_CLASSES), np.float32)
    for c in range(N_CORES):
        out[c * gpc : (c + 1) * gpc, :] = res.results[c]["out"].T
    return out

